# revision 8
# baseline (speedup 1.0000x reference)
"""GAT (4-layer, 4-head) + MLP head on 8 Trainium2 NeuronCores.

Strategy (hardcoded for nn_GATWithDimensionalityReduction_49108656062563):
  - Destination-sharded edge parallelism: core c owns nodes
    [c*SHARD, (c+1)*SHARD) and all edges whose dst falls there; segment
    softmax and scatter-add then need no cross-core reduction.
  - Per layer every core computes the full projected table
    z[n] = [x@W | alpha_src] into two DRAM half-tables (rows fit int16
    indices for the q7 dma_gather). Edges are reordered per 128-node
    block so each chunk's sources live in one half.
  - Edge phase: bulk dma_gather of src rows (<=1024 idx/instr), dst
    alpha via dma_gather from a shard-local table (dst-local idx),
    scores ee = exp(leaky(as+ad)) in-place, one-hot via iota compare,
    and one PSUM-accumulated matmul per chunk yields
    [slot, fo numerators | 4 denominators] per block.
  - Between layers x_{l+1}^T shards are AllGathered (layer-1 x is a
    replicated input).
  - Softmax max-subtraction is skipped: mathematically identical here
    (scores are O(1); no overflow possible).
"""

import math
import time
from contextlib import ExitStack

import numpy as np

import concourse.bass as bass
import concourse.tile as tile
from concourse import bacc, mybir
from concourse.bass_utils import run_bass_kernel_spmd

F32 = mybir.dt.float32
I16 = mybir.dt.int16

N_CORES = 8
H = 4
NEG = -1.0e30
WNI = 1024          # max indices per dma_gather
WCH = WNI // 128    # chunks per gather window

FULL_N = 50000
FULL_LAYERS = ((128, 32), (32, 64), (64, 128), (128, 256))


def _mk_ap(t, offset, dims):
    return bass.AP(t, offset, [list(d) for d in dims])


def _we(fo):
    """table row width: [z(fo) | alpha_s(4)] padded to 64-elem multiple."""
    return ((fo + 4 + 63) // 64) * 64


def _wrap(ids):
    """flat idx list -> [128, n/16] int16 wrapped layout for dma_gather."""
    n = len(ids)
    assert n % 16 == 0
    return np.ascontiguousarray(
        np.tile(ids.reshape(n // 16, 16).T, (8, 1)).astype(np.int16))


# ---------------------------------------------------------------------------
# host-side graph structure
# ---------------------------------------------------------------------------

def build_structure(edge_index, N, NB):
    shard = NB * 128
    n_pad = N_CORES * shard
    half = n_pad // 2
    src = np.concatenate([edge_index[0].astype(np.int64), np.arange(n_pad)])
    dst = np.concatenate([edge_index[1].astype(np.int64), np.arange(n_pad)])
    order = np.argsort(dst, kind="stable")
    src, dst = src[order], dst[order]
    # secondary order: within each block, sources in half A first
    blk = dst >> 7
    half_flag = (src >= half).astype(np.int64)
    order2 = np.lexsort((half_flag, blk))
    src, dst, blk, half_flag = (src[order2], dst[order2], blk[order2],
                                half_flag[order2])

    nblk = n_pad // 128
    cA = np.bincount(blk[half_flag == 0], minlength=nblk)
    cB = np.bincount(blk[half_flag == 1], minlength=nblk)
    CHH = int(math.ceil(max(cA.max(), cB.max()) / 128.0))
    CH2 = 2 * CHH

    tot = nblk * CH2 * 128
    psrc = np.full(tot, -1, np.int64)        # half-local src, dead later
    prel = np.zeros(tot, np.int64)
    pdstl = np.zeros(tot, np.int64)          # shard-local dst
    startsA = np.zeros(nblk, np.int64)
    # positions: A edges of block b at [b*CH2*128 + i], B at + CHH*128 + i
    withinA = np.zeros(len(dst), np.int64)
    withinB = np.zeros(len(dst), np.int64)
    sA = np.zeros(nblk + 1, np.int64); np.cumsum(cA, out=sA[1:])
    sB = np.zeros(nblk + 1, np.int64); np.cumsum(cB, out=sB[1:])
    idxs = np.arange(len(dst))
    isA = half_flag == 0
    # rank within (block, half)
    rank = idxs - np.concatenate(
        [np.repeat(0, 0), np.zeros(len(dst), np.int64)])
    # compute rank via per-block offsets
    block_start = np.searchsorted(blk, np.arange(nblk))
    rank_in_block = idxs - block_start[blk]
    rankA = rank_in_block                     # A edges come first in block
    rankB = rank_in_block - cA[blk]
    pos = np.where(isA,
                   blk * (CH2 * 128) + rankA,
                   blk * (CH2 * 128) + CHH * 128 + rankB)
    psrc[pos] = np.where(isA, src, src - half)
    prel[pos] = dst & 127
    pdstl[pos] = dst % shard

    dead = half                               # dead row index in each half
    psrc[psrc < 0] = dead

    psrc = psrc.reshape(N_CORES, NB, CH2, 128)
    prel = prel.reshape(N_CORES, NB, CH2, 128)
    pdstl = pdstl.reshape(N_CORES, NB, CH2, 128)

    packs = []
    for c in range(N_CORES):
        sc_ = psrc[c]
        idsA = sc_[:, :CHH, :].reshape(-1)    # [NB*CHH*128] A-stream
        idsB = sc_[:, CHH:, :].reshape(-1)
        idsD = pdstl[c].reshape(-1)           # [NB*CH2*128] dst stream
        relc = np.ascontiguousarray(
            prel[c].transpose(2, 0, 1).reshape(128, NB * CH2)
            .astype(np.float32))              # [128, NB*CH2]
        packs.append({
            "zidxA": _wrap(idsA), "zidxB": _wrap(idsB),
            "adidx": _wrap(idsD), "relD": relc,
        })
    return CHH, packs


# ---------------------------------------------------------------------------
# device program
# ---------------------------------------------------------------------------

def build_program(NB, CHH, layers, DB):
    shard = NB * 128
    n_pad = N_CORES * shard
    half = n_pad // 2
    nwin = n_pad // 128
    nlay = len(layers)
    CH2 = 2 * CHH
    assert nwin % DB == 0 and (nwin // 2) % DB == 0

    nc = bacc.Bacc("TRN2", target_bir_lowering=False, debug=False,
                   num_devices=N_CORES)
    fi0 = layers[0][0]
    xT1 = nc.dram_tensor("xT1", [fi0, n_pad], F32, kind="ExternalInput").ap()
    xTmy = nc.dram_tensor("xTmy", [fi0, shard], F32, kind="ExternalInput").ap()
    wcat, wad, brep, deadr = [], [], [], []
    for l, (fi, fo) in enumerate(layers):
        wcat.append(nc.dram_tensor(f"wcat{l}", [fi, fo + 4], F32,
                                   kind="ExternalInput").ap())
        wad.append(nc.dram_tensor(f"wad{l}", [fi, 4], F32,
                                  kind="ExternalInput").ap())
        brep.append(nc.dram_tensor(f"brep{l}", [128, fo], F32,
                                   kind="ExternalInput").ap())
        deadr.append(nc.dram_tensor(f"dead{l}", [1, _we(fo)], F32,
                                    kind="ExternalInput").ap())
    nA = NB * CHH * 128
    nD = NB * CH2 * 128
    zidxA = nc.dram_tensor("zidxA", [128, nA // 16], I16,
                           kind="ExternalInput").ap()
    zidxB = nc.dram_tensor("zidxB", [128, nA // 16], I16,
                           kind="ExternalInput").ap()
    adidx = nc.dram_tensor("adidx", [128, nD // 16], I16,
                           kind="ExternalInput").ap()
    relD = nc.dram_tensor("relD", [128, NB * CH2], F32,
                          kind="ExternalInput").ap()
    iota_in = nc.dram_tensor("iota", [128, 128], F32, kind="ExternalInput").ap()
    ident_in = nc.dram_tensor("ident", [128, 128], F32,
                              kind="ExternalInput").ap()
    wr_in = nc.dram_tensor("wr", [128, 2, 32], F32, kind="ExternalInput").ap()
    wf1_in = nc.dram_tensor("wf1", [32, 64], F32, kind="ExternalInput").ap()
    wf2_in = nc.dram_tensor("wf2", [64, 1], F32, kind="ExternalInput").ap()
    br_in = nc.dram_tensor("brv", [32, 1], F32, kind="ExternalInput").ap()
    bf1_in = nc.dram_tensor("bf1v", [64, 1], F32, kind="ExternalInput").ap()
    bf2_in = nc.dram_tensor("bf2v", [1, 1], F32, kind="ExternalInput").ap()
    out_t = nc.dram_tensor("out", [NB, 128], F32, kind="ExternalOutput").ap()

    zA, zB, adS, agin, agx = [], [], [], [], []
    for l, (fi, fo) in enumerate(layers):
        WE = _we(fo)
        zA.append(nc.dram_tensor(f"zA{l}", [half + 1, WE], F32).ap())
        zB.append(nc.dram_tensor(f"zB{l}", [half + 1, WE], F32).ap())
        adS.append(nc.dram_tensor(f"adS{l}", [shard, 64], F32).ap())
        if l < nlay - 1:
            agin.append(nc.dram_tensor(f"agin{l}", [fo, shard], F32).ap())
            agx.append(nc.dram_tensor(f"agx{l}", [N_CORES, fo, shard],
                                      F32).ap())
        else:
            agin.append(None)
            agx.append(None)

    AT = mybir.ActivationFunctionType
    OP = mybir.AluOpType

    with tile.TileContext(nc) as tc, ExitStack() as ctx:
        cpool = ctx.enter_context(tc.tile_pool(name="consts", bufs=1))
        dlhs = ctx.enter_context(tc.tile_pool(name="dlhs", bufs=3))
        dbat = ctx.enter_context(tc.tile_pool(name="dbat", bufs=3))
        gA_p = ctx.enter_context(tc.tile_pool(name="gA", bufs=3))
        gB_p = ctx.enter_context(tc.tile_pool(name="gB", bufs=3))
        gD_p = ctx.enter_context(tc.tile_pool(name="gD", bufs=3))
        epool = ctx.enter_context(tc.tile_pool(name="edge_small", bufs=3))
        dmat_p = ctx.enter_context(tc.tile_pool(name="dmat", bufs=2))
        ypool = ctx.enter_context(tc.tile_pool(name="epi", bufs=2))
        ps_d = ctx.enter_context(tc.tile_pool(name="ps_dense", bufs=2,
                                              space="PSUM"))
        ps_e = ctx.enter_context(tc.tile_pool(name="ps_edge", bufs=2,
                                              space="PSUM"))
        ps_t = ctx.enter_context(tc.tile_pool(name="ps_tr", bufs=2,
                                              space="PSUM"))

        def csb(ap_in, shape, dtype=F32, tag=None):
            t = cpool.tile(shape, dtype, tag=tag or ap_in.tensor.name)
            nc.sync.dma_start(t[:], ap_in[:])
            return t

        iota_sb = csb(iota_in, [128, 128])
        ident_sb = csb(ident_in, [128, 128])
        zidxA_sb = csb(zidxA, [128, nA // 16], I16)
        zidxB_sb = csb(zidxB, [128, nA // 16], I16)
        adidx_sb = csb(adidx, [128, nD // 16], I16)
        relD_sb = csb(relD, [128, NB * CH2])
        wr_sb = csb(wr_in, [128, 2, 32])
        wf1_sb = csb(wf1_in, [32, 64])
        wf2_sb = csb(wf2_in, [64, 1])
        br_sb = csb(br_in, [32, 1])
        bf1_sb = csb(bf1_in, [64, 1])
        bf2_sb = csb(bf2_in, [1, 1])
        wcat_sb = [csb(wcat[l], [layers[l][0], layers[l][1] + 4])
                   for l in range(nlay)]
        wad_sb = [csb(wad[l], [layers[l][0], 4]) for l in range(nlay)]
        brep_sb = [csb(brep[l], [128, layers[l][1]]) for l in range(nlay)]

        def elu_chain(y_ap, shape, tag):
            m = epool.tile(shape, F32, tag=f"elu_m_{tag}")
            nc.vector.tensor_scalar(m[:], y_ap, 0.0, None, OP.min)
            e = epool.tile(shape, F32, tag=f"elu_e_{tag}")
            nc.scalar.activation(e[:], m[:], AT.Exp)
            nc.vector.tensor_scalar(y_ap, y_ap, 0.0, -1.0, OP.max, OP.add)
            nc.vector.tensor_tensor(y_ap, y_ap, e[:], op=OP.add)

        for l, (fi, fo) in enumerate(layers):
            WE = _we(fo)
            Wz = fo + 4
            C = fo // H
            nfc = (fo + 127) // 128

            # ---------------- dense (replicated, writes zA/zB) ----------
            for wb in range(0, nwin, DB):
                lx = dlhs.tile([fi, DB * 128], F32, tag="lx")
                if l == 0:
                    nc.sync.dma_start(lx[:], xT1[:, wb * 128:(wb + DB) * 128])
                else:
                    s = (wb * 128) // shard
                    woff = wb * 128 - s * shard
                    nc.sync.dma_start(
                        lx[:], agx[l - 1][s, :, woff:woff + DB * 128])
                zb = dbat.tile([128, DB, Wz], F32, tag="zb")
                for j in range(DB):
                    zp = ps_d.tile([128, Wz], F32, tag="zp")
                    nc.tensor.matmul(out=zp[:], lhsT=lx[:, j * 128:(j + 1) * 128],
                                     rhs=wcat_sb[l][:], start=True, stop=True)
                    if j % 2 == 0:
                        nc.vector.tensor_copy(zb[:, j, :], zp[:])
                    else:
                        nc.scalar.activation(zb[:, j, :], zp[:], AT.Copy)
                ztgt = zA[l] if wb < nwin // 2 else zB[l]
                roff = wb * 128 if wb < nwin // 2 else (wb - nwin // 2) * 128
                dst_ap = _mk_ap(ztgt.tensor, roff * WE,
                                [(WE, 128), (128 * WE, DB), (1, Wz)])
                nc.sync.dma_start(dst_ap, zb[:, :, :])
            nc.gpsimd.dma_start(zA[l][half:half + 1, :], deadr[l][:])
            nc.gpsimd.dma_start(zB[l][half:half + 1, :], deadr[l][:])

            # ---------------- mini-dense: alpha_dst for MY shard --------
            for wb in range(0, NB, DB):
                lx = dlhs.tile([fi, DB * 128], F32, tag="lxmy")
                if l == 0:
                    nc.sync.dma_start(lx[:], xTmy[:, wb * 128:(wb + DB) * 128])
                else:
                    nc.sync.dma_start(
                        lx[:], agin[l - 1][:, wb * 128:(wb + DB) * 128])
                ab = dbat.tile([128, DB, 4], F32, tag="ab")
                for j in range(DB):
                    ap_ = ps_d.tile([128, 4], F32, tag="zp")
                    nc.tensor.matmul(out=ap_[:], lhsT=lx[:, j * 128:(j + 1) * 128],
                                     rhs=wad_sb[l][:], start=True, stop=True)
                    nc.vector.tensor_copy(ab[:, j, :], ap_[:])
                dst_ap = _mk_ap(adS[l].tensor, wb * 128 * 64,
                                [(64, 128), (128 * 64, DB), (1, 4)])
                nc.sync.dma_start(dst_ap, ab[:, :, :])

            # ---------------- edge phase --------------------------------
            zwinA, zwinB, dwin = {}, {}, {}

            def zchunk(stream, pos):
                tabs, idxs, wins, pool, we, tagp = (
                    (zA[l], zidxA_sb, zwinA, gA_p, WE, "wa")
                    if stream == "A" else
                    (zB[l], zidxB_sb, zwinB, gB_p, WE, "wb")
                    if stream == "B" else
                    (adS[l], adidx_sb, dwin, gD_p, 64, "wd"))
                w, slot = pos // WCH, pos % WCH
                if w not in wins:
                    total = nA // 128 if stream in "AB" else nD // 128
                    ni = min(WNI, (total - w * WCH) * 128)
                    t = pool.tile([128, WCH, we], F32, tag=tagp)
                    nc.gpsimd.dma_gather(
                        out_ap=t[:, :ni // 128, :], in_ap=tabs[:],
                        idxs_ap=idxs[:, 64 * w: 64 * w + ni // 16],
                        num_idxs=ni, num_idxs_reg=ni, elem_size=we)
                    wins[w] = t
                return wins[w], slot

            for b in range(NB):
                # gather tiles + segment list (contiguous runs in same tiles)
                chunks = []
                for j in range(CH2):
                    st = "A" if j < CHH else "B"
                    zpos = b * CHH + (j % CHH)
                    zt, zs = zchunk(st, zpos)
                    dt_, ds = zchunk("D", b * CH2 + j)
                    chunks.append((zt, zs, dt_, ds))
                segs = []
                j0 = 0
                for j in range(1, CH2 + 1):
                    if (j == CH2 or chunks[j][0] is not chunks[j - 1][0]
                            or chunks[j][1] != chunks[j - 1][1] + 1
                            or chunks[j][2] is not chunks[j - 1][2]
                            or chunks[j][3] != chunks[j - 1][3] + 1):
                        segs.append((j0, j))
                        j0 = j
                # scores ee = exp(max(s, .2s)) written into alpha_s cols
                for (a, bb) in segs:
                    ns = bb - a
                    zt, zs, dt_, ds = chunks[a]
                    asv = zt[:, zs:zs + ns, fo:Wz]
                    adv = dt_[:, ds:ds + ns, 0:4]
                    sc = epool.tile([128, WCH, 4], F32, tag="sc")
                    nc.vector.tensor_tensor(sc[:, :ns, :], asv, adv, op=OP.add)
                    sc2 = epool.tile([128, WCH, 4], F32, tag="sc2")
                    nc.vector.tensor_scalar(sc2[:, :ns, :], sc[:, :ns, :],
                                            0.2, None, OP.mult)
                    nc.vector.tensor_tensor(sc[:, :ns, :], sc[:, :ns, :],
                                            sc2[:, :ns, :], op=OP.max)
                    nc.scalar.activation(asv, sc[:, :ns, :], AT.Exp)
                    # msg scaling in-place on z cols
                    for hf in range(nfc):
                        h0 = hf * 128 // C
                        nh = min(H - h0, 128 // C)
                        cols = zt[:, zs:zs + ns, hf * 128:hf * 128 + nh * C]
                        g4 = bass.AP(cols.tensor, cols.offset,
                                     [cols.ap[0], cols.ap[1], [C, nh], [1, C]])
                        ee4 = asv[:, :, h0:h0 + nh].unsqueeze(3).to_broadcast(
                            [128, ns, nh, C])
                        nc.vector.tensor_tensor(g4, g4, ee4, op=OP.mult)
                # one-hot for the whole block
                dm = dmat_p.tile([128, CH2, 128], F32, tag="dm")
                rel = relD_sb[:, b * CH2:(b + 1) * CH2]
                nc.vector.tensor_tensor(
                    dm[:], rel.unsqueeze(2).to_broadcast([128, CH2, 128]),
                    iota_sb[:].unsqueeze(1).to_broadcast([128, CH2, 128]),
                    op=OP.is_equal)
                # aggregation
                pb = ps_e.tile([128, Wz], F32, tag="pb")
                for j in range(CH2):
                    zt, zs, _, _ = chunks[j]
                    nc.tensor.matmul(out=pb[:], lhsT=dm[:, j, :],
                                     rhs=zt[:, zs, 0:Wz], start=(j == 0),
                                     stop=(j == CH2 - 1))
                # epilogue: x = elu(num/den + b)
                r4 = epool.tile([128, 4], F32, tag="r4")
                nc.vector.reciprocal(r4[:], pb[:, fo:Wz])
                y = ypool.tile([128, fo], F32, tag="y")
                y3 = bass.AP(y[:].tensor, y[:].offset,
                             [y[:].ap[0], [C, H], [1, C]])
                pb3 = bass.AP(pb[:].tensor, pb[:].offset,
                              [pb[:].ap[0], [C, H], [1, C]])
                nc.vector.tensor_tensor(
                    y3, pb3, r4[:].unsqueeze(2).to_broadcast([128, H, C]),
                    op=OP.mult)
                nc.vector.tensor_tensor(y[:], y[:], brep_sb[l][:, :], op=OP.add)
                elu_chain(y[:], [128, fo], "blk")
                xbt = []
                for fc in range(nfc):
                    ncol = min(128, fo - fc * 128)
                    tp = ps_t.tile([128, 128], F32, tag="tp")
                    nc.tensor.transpose(
                        out=tp[:ncol, :], in_=y[:, fc * 128:fc * 128 + ncol],
                        identity=ident_sb[:])
                    xt = ypool.tile([128, 128], F32, tag=f"xt{fc}")
                    if fc % 2 == 0:
                        nc.vector.tensor_copy(xt[:ncol, :], tp[:ncol, :])
                    else:
                        nc.scalar.activation(xt[:ncol, :], tp[:ncol, :], AT.Copy)
                    xbt.append((xt, ncol))
                if l < nlay - 1:
                    for fc, (xt, ncol) in enumerate(xbt):
                        nc.scalar.dma_start(
                            agin[l][fc * 128:fc * 128 + ncol,
                                    b * 128:(b + 1) * 128], xt[:ncol, :])
                else:
                    r1p = ps_t.tile([32, 128], F32, tag="mlp")
                    for fc, (xt, ncol) in enumerate(xbt):
                        nc.tensor.matmul(out=r1p[:], lhsT=wr_sb[:ncol, fc, :],
                                         rhs=xt[:ncol, :], start=(fc == 0),
                                         stop=(fc == len(xbt) - 1))
                    r1 = ypool.tile([32, 128], F32, tag="r1")
                    nc.vector.tensor_scalar(r1[:], r1p[:], br_sb[:, 0:1],
                                            None, OP.add)
                    elu_chain(r1[:], [32, 128], "mlp1")
                    r2p = ps_t.tile([64, 128], F32, tag="mlp")
                    nc.tensor.matmul(out=r2p[:], lhsT=wf1_sb[:], rhs=r1[:],
                                     start=True, stop=True)
                    r2 = ypool.tile([64, 128], F32, tag="r2")
                    nc.vector.tensor_scalar(r2[:], r2p[:], bf1_sb[:, 0:1],
                                            None, OP.add)
                    elu_chain(r2[:], [64, 128], "mlp2")
                    op_ = ps_t.tile([1, 128], F32, tag="mlp")
                    nc.tensor.matmul(out=op_[:], lhsT=wf2_sb[:], rhs=r2[:],
                                     start=True, stop=True)
                    ob = ypool.tile([1, 128], F32, tag="ob")
                    nc.vector.tensor_scalar(ob[:], op_[:], bf2_sb[:, 0:1],
                                            None, OP.add)
                    nc.sync.dma_start(out_t[b:b + 1, :], ob[:])

            if l < nlay - 1:
                nc.gpsimd.collective_compute(
                    "AllGather", OP.bypass,
                    replica_groups=[list(range(N_CORES))],
                    ins=[agin[l][:]], outs=[agx[l][:]])

    nc.compile()
    return nc


# ---------------------------------------------------------------------------
# host orchestration
# ---------------------------------------------------------------------------

def _prep_weights(layers, W, A_s, A_d, B):
    wcat, wadl, brep, dead = [], [], [], []
    for l, (fi, fo) in enumerate(layers):
        C = fo // H
        As = np.zeros((fo, 4), np.float32)
        Ad = np.zeros((fo, 4), np.float32)
        for h in range(H):
            As[h * C:(h + 1) * C, h] = A_s[l][h]
            Ad[h * C:(h + 1) * C, h] = A_d[l][h]
        wcat.append(np.ascontiguousarray(
            np.concatenate([W[l], W[l] @ As], axis=1), np.float32))
        wadl.append(np.ascontiguousarray(W[l] @ Ad, np.float32))
        brep.append(np.ascontiguousarray(
            np.tile(B[l][None, :], (128, 1)), np.float32))
        d = np.zeros((1, _we(fo)), np.float32)
        d[0, fo:fo + 4] = NEG
        dead.append(d)
    return wcat, wadl, brep, dead


_CACHE = {}


def run_gat(x, edge_index, W, A_s, A_d, B, Wr, br, Wf1, bf1, Wf2, bf2,
            layers, N, NB, DB=7):
    shard = NB * 128
    n_pad = N_CORES * shard
    CHH, packs = build_structure(edge_index, N, NB)

    key = (NB, CHH, tuple(layers), DB)
    if key not in _CACHE:
        t0 = time.time()
        _CACHE[key] = build_program(NB, CHH, layers, DB=DB)
        print(f"[kernel] built+compiled program NB={NB} CHH={CHH} "
              f"in {time.time() - t0:.1f}s", flush=True)
    nc = _CACHE[key]

    wcat, wadl, brep, dead = _prep_weights(layers, W, A_s, A_d, B)
    xp = np.zeros((n_pad, layers[0][0]), np.float32)
    xp[:N] = x
    xT1 = np.ascontiguousarray(xp.T)
    base = {
        "xT1": xT1,
        "iota": np.ascontiguousarray(
            np.tile(np.arange(128, dtype=np.float32)[None, :], (128, 1))),
        "ident": np.eye(128, dtype=np.float32),
        "wr": np.ascontiguousarray(
            Wr.astype(np.float32).reshape(2, 128, 32).transpose(1, 0, 2)),
        "wf1": Wf1.astype(np.float32), "wf2": Wf2.astype(np.float32),
        "brv": br.reshape(32, 1).astype(np.float32),
        "bf1v": bf1.reshape(64, 1).astype(np.float32),
        "bf2v": bf2.reshape(1, 1).astype(np.float32),
    }
    for l in range(len(layers)):
        base[f"wcat{l}"] = wcat[l]
        base[f"wad{l}"] = wadl[l]
        base[f"brep{l}"] = brep[l]
        base[f"dead{l}"] = dead[l]
    in_maps = []
    for c in range(N_CORES):
        m = dict(base)
        m.update(packs[c])
        m["xTmy"] = np.ascontiguousarray(xT1[:, c * shard:(c + 1) * shard])
        in_maps.append(m)

    res = run_bass_kernel_spmd(nc, in_maps, list(range(N_CORES)))
    outs = [res.results[c]["out"].reshape(-1) for c in range(N_CORES)]
    full = np.concatenate(outs)[:N]
    return full.reshape(N, 1).astype(np.float32)


def kernel(**inputs):
    x = np.asarray(inputs["x"], np.float32)
    edge_index = np.asarray(inputs["edge_index"], np.int32)
    W = [np.asarray(inputs[f"W{l+1}"], np.float32) for l in range(4)]
    A_s = [np.asarray(inputs[f"as{l+1}"], np.float32) for l in range(4)]
    A_d = [np.asarray(inputs[f"ad{l+1}"], np.float32) for l in range(4)]
    B = [np.asarray(inputs[f"b{l+1}"], np.float32) for l in range(4)]
    return run_gat(
        x, edge_index, W, A_s, A_d, B,
        np.asarray(inputs["Wr"], np.float32), np.asarray(inputs["br"], np.float32),
        np.asarray(inputs["Wf1"], np.float32), np.asarray(inputs["bf1"], np.float32),
        np.asarray(inputs["Wf2"], np.float32), np.asarray(inputs["bf2"], np.float32),
        FULL_LAYERS, FULL_N, NB=49)


# revision 10
# speedup vs baseline: 2.4680x; 2.4680x over previous
"""GAT (4-layer, 4-head) + MLP head on 8 Trainium2 NeuronCores.

Strategy (hardcoded for nn_GATWithDimensionalityReduction_49108656062563):
  - Destination-sharded edge parallelism: core c owns nodes
    [c*SHARD, (c+1)*SHARD) and all edges whose dst falls there; segment
    softmax and scatter-add then need no cross-core reduction.
  - Per layer every core computes the full projected table
    z[n] = [x@W | alpha_src] into two DRAM half-tables (rows fit int16
    indices for the q7 dma_gather). Edges are reordered per 128-node
    block so each chunk's sources live in one half.
  - Edge phase: bulk dma_gather of src rows (<=1024 idx/instr), dst
    alpha via dma_gather from a shard-local table (dst-local idx),
    scores ee = exp(leaky(as+ad)) in-place, one-hot via iota compare,
    and one PSUM-accumulated matmul per chunk yields
    [slot, fo numerators | 4 denominators] per block.
  - Between layers x_{l+1}^T shards are AllGathered (layer-1 x is a
    replicated input).
  - Softmax max-subtraction is skipped: mathematically identical here
    (scores are O(1); no overflow possible).
"""

import math
import time
from contextlib import ExitStack

import numpy as np

import concourse.bass as bass
import concourse.tile as tile
from concourse import bacc, mybir
from concourse.bass_utils import run_bass_kernel_spmd

F32 = mybir.dt.float32
I16 = mybir.dt.int16

N_CORES = 8
H = 4
NEG = -1.0e30
WNI = 1024          # max indices per dma_gather
WCH = WNI // 128    # chunks per gather window

FULL_N = 50000
FULL_LAYERS = ((128, 32), (32, 64), (64, 128), (128, 256))


def _mk_ap(t, offset, dims):
    return bass.AP(t, offset, [list(d) for d in dims])


def _we(fo):
    """table row width: [z(fo) | alpha_s(4)] padded to 64-elem multiple."""
    return ((fo + 4 + 63) // 64) * 64


def _wrap(ids):
    """flat idx list -> [128, n/16] int16 wrapped layout for dma_gather."""
    n = len(ids)
    assert n % 16 == 0
    return np.ascontiguousarray(
        np.tile(ids.reshape(n // 16, 16).T, (8, 1)).astype(np.int16))


# ---------------------------------------------------------------------------
# host-side graph structure
# ---------------------------------------------------------------------------

def build_structure(edge_index, N, NB):
    shard = NB * 128
    n_pad = N_CORES * shard
    half = n_pad // 2
    src = np.concatenate([edge_index[0].astype(np.int64), np.arange(n_pad)])
    dst = np.concatenate([edge_index[1].astype(np.int64), np.arange(n_pad)])
    order = np.argsort(dst, kind="stable")
    src, dst = src[order], dst[order]
    # secondary order: within each block, sources in half A first
    blk = dst >> 7
    half_flag = (src >= half).astype(np.int64)
    order2 = np.lexsort((half_flag, blk))
    src, dst, blk, half_flag = (src[order2], dst[order2], blk[order2],
                                half_flag[order2])

    nblk = n_pad // 128
    cA = np.bincount(blk[half_flag == 0], minlength=nblk)
    cB = np.bincount(blk[half_flag == 1], minlength=nblk)
    CHH = int(math.ceil(max(cA.max(), cB.max()) / 128.0))
    CH2 = 2 * CHH

    tot = nblk * CH2 * 128
    psrc = np.full(tot, -1, np.int64)        # half-local src, dead later
    prel = np.zeros(tot, np.int64)
    pdstl = np.zeros(tot, np.int64)          # shard-local dst
    startsA = np.zeros(nblk, np.int64)
    # positions: A edges of block b at [b*CH2*128 + i], B at + CHH*128 + i
    withinA = np.zeros(len(dst), np.int64)
    withinB = np.zeros(len(dst), np.int64)
    sA = np.zeros(nblk + 1, np.int64); np.cumsum(cA, out=sA[1:])
    sB = np.zeros(nblk + 1, np.int64); np.cumsum(cB, out=sB[1:])
    idxs = np.arange(len(dst))
    isA = half_flag == 0
    # rank within (block, half)
    rank = idxs - np.concatenate(
        [np.repeat(0, 0), np.zeros(len(dst), np.int64)])
    # compute rank via per-block offsets
    block_start = np.searchsorted(blk, np.arange(nblk))
    rank_in_block = idxs - block_start[blk]
    rankA = rank_in_block                     # A edges come first in block
    rankB = rank_in_block - cA[blk]
    pos = np.where(isA,
                   blk * (CH2 * 128) + rankA,
                   blk * (CH2 * 128) + CHH * 128 + rankB)
    psrc[pos] = np.where(isA, src, src - half)
    prel[pos] = dst & 127
    pdstl[pos] = dst % shard

    dead = half                               # dead row index in each half
    psrc[psrc < 0] = dead

    psrc = psrc.reshape(N_CORES, NB, CH2, 128)
    prel = prel.reshape(N_CORES, NB, CH2, 128)
    pdstl = pdstl.reshape(N_CORES, NB, CH2, 128)

    packs = []
    for c in range(N_CORES):
        sc_ = psrc[c]
        idsA = sc_[:, :CHH, :].reshape(-1)    # [NB*CHH*128] A-stream
        idsB = sc_[:, CHH:, :].reshape(-1)
        idsD = pdstl[c].reshape(-1)           # [NB*CH2*128] dst stream
        relc = np.ascontiguousarray(
            prel[c].transpose(2, 0, 1).reshape(128, NB * CH2)
            .astype(np.float32))              # [128, NB*CH2]
        packs.append({
            "zidxA": _wrap(idsA), "zidxB": _wrap(idsB),
            "adidx": _wrap(idsD), "relD": relc,
        })
    return CHH, packs


# ---------------------------------------------------------------------------
# device program
# ---------------------------------------------------------------------------

def build_program(NB, CHH, layers, DB, reps=1):
    shard = NB * 128
    n_pad = N_CORES * shard
    half = n_pad // 2
    nwin = n_pad // 128
    nlay = len(layers)
    CH2 = 2 * CHH
    assert nwin % DB == 0 and (nwin // 2) % DB == 0

    nc = bacc.Bacc("TRN2", target_bir_lowering=False, debug=False,
                   num_devices=N_CORES)
    fi0 = layers[0][0]
    xT1 = nc.dram_tensor("xT1", [fi0, n_pad], F32, kind="ExternalInput").ap()
    xTmy = nc.dram_tensor("xTmy", [fi0, shard], F32, kind="ExternalInput").ap()
    wcat, wad, brep, deadr = [], [], [], []
    for l, (fi, fo) in enumerate(layers):
        wcat.append(nc.dram_tensor(f"wcat{l}", [fi, fo + 4], F32,
                                   kind="ExternalInput").ap())
        wad.append(nc.dram_tensor(f"wad{l}", [fi, 4], F32,
                                  kind="ExternalInput").ap())
        brep.append(nc.dram_tensor(f"brep{l}", [128, fo], F32,
                                   kind="ExternalInput").ap())
        deadr.append(nc.dram_tensor(f"dead{l}", [1, _we(fo)], F32,
                                    kind="ExternalInput").ap())
    nA = NB * CHH * 128
    nD = NB * CH2 * 128
    zidxA = nc.dram_tensor("zidxA", [128, nA // 16], I16,
                           kind="ExternalInput").ap()
    zidxB = nc.dram_tensor("zidxB", [128, nA // 16], I16,
                           kind="ExternalInput").ap()
    adidx = nc.dram_tensor("adidx", [128, nD // 16], I16,
                           kind="ExternalInput").ap()
    relD = nc.dram_tensor("relD", [128, NB * CH2], F32,
                          kind="ExternalInput").ap()
    iota_in = nc.dram_tensor("iota", [128, 128], F32, kind="ExternalInput").ap()
    ident_in = nc.dram_tensor("ident", [128, 128], F32,
                              kind="ExternalInput").ap()
    wr_in = nc.dram_tensor("wr", [128, 2, 32], F32, kind="ExternalInput").ap()
    wf1_in = nc.dram_tensor("wf1", [32, 64], F32, kind="ExternalInput").ap()
    wf2_in = nc.dram_tensor("wf2", [64, 1], F32, kind="ExternalInput").ap()
    br_in = nc.dram_tensor("brv", [32, 1], F32, kind="ExternalInput").ap()
    bf1_in = nc.dram_tensor("bf1v", [64, 1], F32, kind="ExternalInput").ap()
    bf2_in = nc.dram_tensor("bf2v", [1, 1], F32, kind="ExternalInput").ap()
    out_t = nc.dram_tensor("out", [NB, 128], F32, kind="ExternalOutput").ap()

    zA, zB, adS, agin, agx = [], [], [], [], []
    for l, (fi, fo) in enumerate(layers):
        WE = _we(fo)
        zA.append(nc.dram_tensor(f"zA{l}", [half + 1, WE], F32).ap())
        zB.append(nc.dram_tensor(f"zB{l}", [half + 1, WE], F32).ap())
        adS.append(nc.dram_tensor(f"adS{l}", [shard, 64], F32).ap())
        if l < nlay - 1:
            agin.append(nc.dram_tensor(f"agin{l}", [fo, shard], F32).ap())
            agx.append(nc.dram_tensor(f"agx{l}", [N_CORES, fo, shard],
                                      F32).ap())
        else:
            agin.append(None)
            agx.append(None)

    AT = mybir.ActivationFunctionType
    OP = mybir.AluOpType

    with tile.TileContext(nc) as tc, ExitStack() as ctx:
        cpool = ctx.enter_context(tc.tile_pool(name="consts", bufs=1))
        dlhs = ctx.enter_context(tc.tile_pool(name="dlhs", bufs=3))
        dbat = ctx.enter_context(tc.tile_pool(name="dbat", bufs=3))
        gA_p = ctx.enter_context(tc.tile_pool(name="gA", bufs=3))
        gB_p = ctx.enter_context(tc.tile_pool(name="gB", bufs=3))
        gD_p = ctx.enter_context(tc.tile_pool(name="gD", bufs=3))
        epool = ctx.enter_context(tc.tile_pool(name="edge_small", bufs=3))
        dmat_p = ctx.enter_context(tc.tile_pool(name="dmat", bufs=2))
        ypool = ctx.enter_context(tc.tile_pool(name="epi", bufs=2))
        ps_d = ctx.enter_context(tc.tile_pool(name="ps_dense", bufs=2,
                                              space="PSUM"))
        ps_e = ctx.enter_context(tc.tile_pool(name="ps_edge", bufs=2,
                                              space="PSUM"))
        ps_t = ctx.enter_context(tc.tile_pool(name="ps_tr", bufs=2,
                                              space="PSUM"))

        def csb(ap_in, shape, dtype=F32, tag=None):
            t = cpool.tile(shape, dtype, tag=tag or ap_in.tensor.name)
            nc.sync.dma_start(t[:], ap_in[:])
            return t

        iota_sb = csb(iota_in, [128, 128])
        ident_sb = csb(ident_in, [128, 128])
        zidxA_sb = csb(zidxA, [128, nA // 16], I16)
        zidxB_sb = csb(zidxB, [128, nA // 16], I16)
        adidx_sb = csb(adidx, [128, nD // 16], I16)
        relD_sb = csb(relD, [128, NB * CH2])
        wr_sb = csb(wr_in, [128, 2, 32])
        wf1_sb = csb(wf1_in, [32, 64])
        wf2_sb = csb(wf2_in, [64, 1])
        br_sb = csb(br_in, [32, 1])
        bf1_sb = csb(bf1_in, [64, 1])
        bf2_sb = csb(bf2_in, [1, 1])
        wcat_sb = [csb(wcat[l], [layers[l][0], layers[l][1] + 4])
                   for l in range(nlay)]
        wad_sb = [csb(wad[l], [layers[l][0], 4]) for l in range(nlay)]
        brep_sb = [csb(brep[l], [128, layers[l][1]]) for l in range(nlay)]

        def elu_chain(y_ap, shape, tag):
            m = epool.tile(shape, F32, tag=f"elu_m_{tag}")
            nc.vector.tensor_scalar(m[:], y_ap, 0.0, None, OP.min)
            e = epool.tile(shape, F32, tag=f"elu_e_{tag}")
            nc.scalar.activation(e[:], m[:], AT.Exp)
            nc.vector.tensor_scalar(y_ap, y_ap, 0.0, -1.0, OP.max, OP.add)
            nc.vector.tensor_tensor(y_ap, y_ap, e[:], op=OP.add)

        for _rep in range(reps):
          for l, (fi, fo) in enumerate(layers):
            WE = _we(fo)
            Wz = fo + 4
            C = fo // H
            nfc = (fo + 127) // 128

            # ---------------- dense (replicated, writes zA/zB) ----------
            for wb in range(0, nwin, DB):
                lx = dlhs.tile([fi, DB * 128], F32, tag="lx")
                if l == 0:
                    nc.sync.dma_start(lx[:], xT1[:, wb * 128:(wb + DB) * 128])
                else:
                    s = (wb * 128) // shard
                    woff = wb * 128 - s * shard
                    nc.sync.dma_start(
                        lx[:], agx[l - 1][s, :, woff:woff + DB * 128])
                zb = dbat.tile([128, DB, Wz], F32, tag="zb")
                for j in range(DB):
                    zp = ps_d.tile([128, Wz], F32, tag="zp")
                    nc.tensor.matmul(out=zp[:], lhsT=lx[:, j * 128:(j + 1) * 128],
                                     rhs=wcat_sb[l][:], start=True, stop=True)
                    if j % 2 == 0:
                        nc.vector.tensor_copy(zb[:, j, :], zp[:])
                    else:
                        nc.scalar.activation(zb[:, j, :], zp[:], AT.Copy)
                ztgt = zA[l] if wb < nwin // 2 else zB[l]
                roff = wb * 128 if wb < nwin // 2 else (wb - nwin // 2) * 128
                dst_ap = _mk_ap(ztgt.tensor, roff * WE,
                                [(WE, 128), (128 * WE, DB), (1, Wz)])
                nc.sync.dma_start(dst_ap, zb[:, :, :])
            nc.gpsimd.dma_start(zA[l][half:half + 1, :], deadr[l][:])
            nc.gpsimd.dma_start(zB[l][half:half + 1, :], deadr[l][:])

            # ---------------- mini-dense: alpha_dst for MY shard --------
            for wb in range(0, NB, DB):
                lx = dlhs.tile([fi, DB * 128], F32, tag="lxmy")
                if l == 0:
                    nc.sync.dma_start(lx[:], xTmy[:, wb * 128:(wb + DB) * 128])
                else:
                    nc.sync.dma_start(
                        lx[:], agin[l - 1][:, wb * 128:(wb + DB) * 128])
                ab = dbat.tile([128, DB, 4], F32, tag="ab")
                for j in range(DB):
                    ap_ = ps_d.tile([128, 4], F32, tag="zp")
                    nc.tensor.matmul(out=ap_[:], lhsT=lx[:, j * 128:(j + 1) * 128],
                                     rhs=wad_sb[l][:], start=True, stop=True)
                    nc.vector.tensor_copy(ab[:, j, :], ap_[:])
                dst_ap = _mk_ap(adS[l].tensor, wb * 128 * 64,
                                [(64, 128), (128 * 64, DB), (1, 4)])
                nc.sync.dma_start(dst_ap, ab[:, :, :])

            # ---------------- edge phase --------------------------------
            zwinA, zwinB, dwin = {}, {}, {}

            def zchunk(stream, pos):
                tabs, idxs, wins, pool, we, tagp = (
                    (zA[l], zidxA_sb, zwinA, gA_p, WE, "wa")
                    if stream == "A" else
                    (zB[l], zidxB_sb, zwinB, gB_p, WE, "wb")
                    if stream == "B" else
                    (adS[l], adidx_sb, dwin, gD_p, 64, "wd"))
                w, slot = pos // WCH, pos % WCH
                if w not in wins:
                    total = nA // 128 if stream in "AB" else nD // 128
                    ni = min(WNI, (total - w * WCH) * 128)
                    t = pool.tile([128, WCH, we], F32, tag=tagp)
                    nc.gpsimd.dma_gather(
                        out_ap=t[:, :ni // 128, :], in_ap=tabs[:],
                        idxs_ap=idxs[:, 64 * w: 64 * w + ni // 16],
                        num_idxs=ni, num_idxs_reg=ni, elem_size=we)
                    wins[w] = t
                return wins[w], slot

            for b in range(NB):
                # gather tiles + segment list (contiguous runs in same tiles)
                chunks = []
                for j in range(CH2):
                    st = "A" if j < CHH else "B"
                    zpos = b * CHH + (j % CHH)
                    zt, zs = zchunk(st, zpos)
                    dt_, ds = zchunk("D", b * CH2 + j)
                    chunks.append((zt, zs, dt_, ds))
                segs = []
                j0 = 0
                for j in range(1, CH2 + 1):
                    if (j == CH2 or chunks[j][0] is not chunks[j - 1][0]
                            or chunks[j][1] != chunks[j - 1][1] + 1
                            or chunks[j][2] is not chunks[j - 1][2]
                            or chunks[j][3] != chunks[j - 1][3] + 1):
                        segs.append((j0, j))
                        j0 = j
                # scores ee = exp(max(s, .2s)) written into alpha_s cols
                for (a, bb) in segs:
                    ns = bb - a
                    zt, zs, dt_, ds = chunks[a]
                    asv = zt[:, zs:zs + ns, fo:Wz]
                    adv = dt_[:, ds:ds + ns, 0:4]
                    sc = epool.tile([128, WCH, 4], F32, tag="sc")
                    nc.vector.tensor_tensor(sc[:, :ns, :], asv, adv, op=OP.add)
                    sc2 = epool.tile([128, WCH, 4], F32, tag="sc2")
                    nc.vector.tensor_scalar(sc2[:, :ns, :], sc[:, :ns, :],
                                            0.2, None, OP.mult)
                    nc.vector.tensor_tensor(sc[:, :ns, :], sc[:, :ns, :],
                                            sc2[:, :ns, :], op=OP.max)
                    nc.scalar.activation(asv, sc[:, :ns, :], AT.Exp)
                    # msg scaling in-place on z cols
                    for hf in range(nfc):
                        h0 = hf * 128 // C
                        nh = min(H - h0, 128 // C)
                        cols = zt[:, zs:zs + ns, hf * 128:hf * 128 + nh * C]
                        g4 = bass.AP(cols.tensor, cols.offset,
                                     [cols.ap[0], cols.ap[1], [C, nh], [1, C]])
                        ee4 = asv[:, :, h0:h0 + nh].unsqueeze(3).to_broadcast(
                            [128, ns, nh, C])
                        nc.vector.tensor_tensor(g4, g4, ee4, op=OP.mult)
                # one-hot for the whole block
                dm = dmat_p.tile([128, CH2, 128], F32, tag="dm")
                rel = relD_sb[:, b * CH2:(b + 1) * CH2]
                nc.vector.tensor_tensor(
                    dm[:], rel.unsqueeze(2).to_broadcast([128, CH2, 128]),
                    iota_sb[:].unsqueeze(1).to_broadcast([128, CH2, 128]),
                    op=OP.is_equal)
                # aggregation
                pb = ps_e.tile([128, Wz], F32, tag="pb")
                for j in range(CH2):
                    zt, zs, _, _ = chunks[j]
                    nc.tensor.matmul(out=pb[:], lhsT=dm[:, j, :],
                                     rhs=zt[:, zs, 0:Wz], start=(j == 0),
                                     stop=(j == CH2 - 1))
                # epilogue: x = elu(num/den + b)
                r4 = epool.tile([128, 4], F32, tag="r4")
                nc.vector.reciprocal(r4[:], pb[:, fo:Wz])
                y = ypool.tile([128, fo], F32, tag="y")
                y3 = bass.AP(y[:].tensor, y[:].offset,
                             [y[:].ap[0], [C, H], [1, C]])
                pb3 = bass.AP(pb[:].tensor, pb[:].offset,
                              [pb[:].ap[0], [C, H], [1, C]])
                nc.vector.tensor_tensor(
                    y3, pb3, r4[:].unsqueeze(2).to_broadcast([128, H, C]),
                    op=OP.mult)
                nc.vector.tensor_tensor(y[:], y[:], brep_sb[l][:, :], op=OP.add)
                elu_chain(y[:], [128, fo], "blk")
                xbt = []
                for fc in range(nfc):
                    ncol = min(128, fo - fc * 128)
                    tp = ps_t.tile([128, 128], F32, tag="tp")
                    nc.tensor.transpose(
                        out=tp[:ncol, :], in_=y[:, fc * 128:fc * 128 + ncol],
                        identity=ident_sb[:])
                    xt = ypool.tile([128, 128], F32, tag=f"xt{fc}")
                    if fc % 2 == 0:
                        nc.vector.tensor_copy(xt[:ncol, :], tp[:ncol, :])
                    else:
                        nc.scalar.activation(xt[:ncol, :], tp[:ncol, :], AT.Copy)
                    xbt.append((xt, ncol))
                if l < nlay - 1:
                    for fc, (xt, ncol) in enumerate(xbt):
                        nc.scalar.dma_start(
                            agin[l][fc * 128:fc * 128 + ncol,
                                    b * 128:(b + 1) * 128], xt[:ncol, :])
                else:
                    r1p = ps_t.tile([32, 128], F32, tag="mlp")
                    for fc, (xt, ncol) in enumerate(xbt):
                        nc.tensor.matmul(out=r1p[:], lhsT=wr_sb[:ncol, fc, :],
                                         rhs=xt[:ncol, :], start=(fc == 0),
                                         stop=(fc == len(xbt) - 1))
                    r1 = ypool.tile([32, 128], F32, tag="r1")
                    nc.vector.tensor_scalar(r1[:], r1p[:], br_sb[:, 0:1],
                                            None, OP.add)
                    elu_chain(r1[:], [32, 128], "mlp1")
                    r2p = ps_t.tile([64, 128], F32, tag="mlp")
                    nc.tensor.matmul(out=r2p[:], lhsT=wf1_sb[:], rhs=r1[:],
                                     start=True, stop=True)
                    r2 = ypool.tile([64, 128], F32, tag="r2")
                    nc.vector.tensor_scalar(r2[:], r2p[:], bf1_sb[:, 0:1],
                                            None, OP.add)
                    elu_chain(r2[:], [64, 128], "mlp2")
                    op_ = ps_t.tile([1, 128], F32, tag="mlp")
                    nc.tensor.matmul(out=op_[:], lhsT=wf2_sb[:], rhs=r2[:],
                                     start=True, stop=True)
                    ob = ypool.tile([1, 128], F32, tag="ob")
                    nc.vector.tensor_scalar(ob[:], op_[:], bf2_sb[:, 0:1],
                                            None, OP.add)
                    nc.sync.dma_start(out_t[b:b + 1, :], ob[:])

            if l < nlay - 1:
                nc.gpsimd.collective_compute(
                    "AllGather", OP.bypass,
                    replica_groups=[list(range(N_CORES))],
                    ins=[agin[l][:]], outs=[agx[l][:]])

    nc.compile()
    return nc


# ---------------------------------------------------------------------------
# host orchestration
# ---------------------------------------------------------------------------

def _prep_weights(layers, W, A_s, A_d, B):
    wcat, wadl, brep, dead = [], [], [], []
    for l, (fi, fo) in enumerate(layers):
        C = fo // H
        As = np.zeros((fo, 4), np.float32)
        Ad = np.zeros((fo, 4), np.float32)
        for h in range(H):
            As[h * C:(h + 1) * C, h] = A_s[l][h]
            Ad[h * C:(h + 1) * C, h] = A_d[l][h]
        wcat.append(np.ascontiguousarray(
            np.concatenate([W[l], W[l] @ As], axis=1), np.float32))
        wadl.append(np.ascontiguousarray(W[l] @ Ad, np.float32))
        brep.append(np.ascontiguousarray(
            np.tile(B[l][None, :], (128, 1)), np.float32))
        d = np.zeros((1, _we(fo)), np.float32)
        d[0, fo:fo + 4] = NEG
        dead.append(d)
    return wcat, wadl, brep, dead


_CACHE = {}


def run_gat(x, edge_index, W, A_s, A_d, B, Wr, br, Wf1, bf1, Wf2, bf2,
            layers, N, NB, DB=7, reps=1):
    shard = NB * 128
    n_pad = N_CORES * shard
    CHH, packs = build_structure(edge_index, N, NB)

    key = (NB, CHH, tuple(layers), DB, reps)
    if key not in _CACHE:
        t0 = time.time()
        _CACHE[key] = build_program(NB, CHH, layers, DB=DB, reps=reps)
        print(f"[kernel] built+compiled program NB={NB} CHH={CHH} "
              f"in {time.time() - t0:.1f}s", flush=True)
    nc = _CACHE[key]

    wcat, wadl, brep, dead = _prep_weights(layers, W, A_s, A_d, B)
    xp = np.zeros((n_pad, layers[0][0]), np.float32)
    xp[:N] = x
    xT1 = np.ascontiguousarray(xp.T)
    base = {
        "xT1": xT1,
        "iota": np.ascontiguousarray(
            np.tile(np.arange(128, dtype=np.float32)[None, :], (128, 1))),
        "ident": np.eye(128, dtype=np.float32),
        "wr": np.ascontiguousarray(
            Wr.astype(np.float32).reshape(2, 128, 32).transpose(1, 0, 2)),
        "wf1": Wf1.astype(np.float32), "wf2": Wf2.astype(np.float32),
        "brv": br.reshape(32, 1).astype(np.float32),
        "bf1v": bf1.reshape(64, 1).astype(np.float32),
        "bf2v": bf2.reshape(1, 1).astype(np.float32),
    }
    for l in range(len(layers)):
        base[f"wcat{l}"] = wcat[l]
        base[f"wad{l}"] = wadl[l]
        base[f"brep{l}"] = brep[l]
        base[f"dead{l}"] = dead[l]
    in_maps = []
    for c in range(N_CORES):
        m = dict(base)
        m.update(packs[c])
        m["xTmy"] = np.ascontiguousarray(xT1[:, c * shard:(c + 1) * shard])
        in_maps.append(m)

    res = run_bass_kernel_spmd(nc, in_maps, list(range(N_CORES)))
    outs = [res.results[c]["out"].reshape(-1) for c in range(N_CORES)]
    full = np.concatenate(outs)[:N]
    return full.reshape(N, 1).astype(np.float32)


def kernel(**inputs):
    x = np.asarray(inputs["x"], np.float32)
    edge_index = np.asarray(inputs["edge_index"], np.int32)
    W = [np.asarray(inputs[f"W{l+1}"], np.float32) for l in range(4)]
    A_s = [np.asarray(inputs[f"as{l+1}"], np.float32) for l in range(4)]
    A_d = [np.asarray(inputs[f"ad{l+1}"], np.float32) for l in range(4)]
    B = [np.asarray(inputs[f"b{l+1}"], np.float32) for l in range(4)]
    return run_gat(
        x, edge_index, W, A_s, A_d, B,
        np.asarray(inputs["Wr"], np.float32), np.asarray(inputs["br"], np.float32),
        np.asarray(inputs["Wf1"], np.float32), np.asarray(inputs["bf1"], np.float32),
        np.asarray(inputs["Wf2"], np.float32), np.asarray(inputs["bf2"], np.float32),
        FULL_LAYERS, FULL_N, NB=49)


# revision 11
# speedup vs baseline: 2.6796x; 1.0858x over previous
"""GAT (4-layer, 4-head) + MLP head on 8 Trainium2 NeuronCores.

Strategy (hardcoded for nn_GATWithDimensionalityReduction_49108656062563):
  - Destination-sharded edge parallelism: core c owns nodes
    [c*SHARD, (c+1)*SHARD) and all edges whose dst falls there; segment
    softmax and scatter-add then need no cross-core reduction.
  - Per layer every core computes the full projected table
    z[n] = [x@W | alpha_src] into two DRAM half-tables (rows fit int16
    indices for the q7 dma_gather). Edges are reordered per 128-node
    block so each chunk's sources live in one half.
  - Edge phase: bulk dma_gather of src rows (<=1024 idx/instr), dst
    alpha via dma_gather from a shard-local table (dst-local idx),
    scores ee = exp(leaky(as+ad)) in-place, one-hot via iota compare,
    and one PSUM-accumulated matmul per chunk yields
    [slot, fo numerators | 4 denominators] per block.
  - Between layers x_{l+1}^T shards are AllGathered (layer-1 x is a
    replicated input).
  - Softmax max-subtraction is skipped: mathematically identical here
    (scores are O(1); no overflow possible).
"""

import math
import time
from contextlib import ExitStack

import numpy as np

import concourse.bass as bass
import concourse.tile as tile
from concourse import bacc, mybir
from concourse.bass_utils import run_bass_kernel_spmd

F32 = mybir.dt.float32
I16 = mybir.dt.int16

N_CORES = 8
H = 4
NEG = -1.0e30
WNI = 1024          # max indices per dma_gather
WCH = WNI // 128    # chunks per gather window

FULL_N = 50000
FULL_LAYERS = ((128, 32), (32, 64), (64, 128), (128, 256))


def _mk_ap(t, offset, dims):
    return bass.AP(t, offset, [list(d) for d in dims])


def _we(fo):
    """table row width: [z(fo) | alpha_s(4)] padded to 64-elem multiple."""
    return ((fo + 4 + 63) // 64) * 64


def _wrap(ids):
    """flat idx list -> [128, n/16] int16 wrapped layout for dma_gather."""
    n = len(ids)
    assert n % 16 == 0
    return np.ascontiguousarray(
        np.tile(ids.reshape(n // 16, 16).T, (8, 1)).astype(np.int16))


# ---------------------------------------------------------------------------
# host-side graph structure
# ---------------------------------------------------------------------------

def build_structure(edge_index, N, NB):
    shard = NB * 128
    n_pad = N_CORES * shard
    half = n_pad // 2
    src = np.concatenate([edge_index[0].astype(np.int64), np.arange(n_pad)])
    dst = np.concatenate([edge_index[1].astype(np.int64), np.arange(n_pad)])
    order = np.argsort(dst, kind="stable")
    src, dst = src[order], dst[order]
    # secondary order: within each block, sources in half A first
    blk = dst >> 7
    half_flag = (src >= half).astype(np.int64)
    order2 = np.lexsort((half_flag, blk))
    src, dst, blk, half_flag = (src[order2], dst[order2], blk[order2],
                                half_flag[order2])

    nblk = n_pad // 128
    cA = np.bincount(blk[half_flag == 0], minlength=nblk)
    cB = np.bincount(blk[half_flag == 1], minlength=nblk)
    CHH = int(math.ceil(max(cA.max(), cB.max()) / 128.0))
    CH2 = 2 * CHH

    tot = nblk * CH2 * 128
    psrc = np.full(tot, -1, np.int64)        # half-local src, dead later
    prel = np.zeros(tot, np.int64)
    pdstl = np.zeros(tot, np.int64)          # shard-local dst
    startsA = np.zeros(nblk, np.int64)
    # positions: A edges of block b at [b*CH2*128 + i], B at + CHH*128 + i
    withinA = np.zeros(len(dst), np.int64)
    withinB = np.zeros(len(dst), np.int64)
    sA = np.zeros(nblk + 1, np.int64); np.cumsum(cA, out=sA[1:])
    sB = np.zeros(nblk + 1, np.int64); np.cumsum(cB, out=sB[1:])
    idxs = np.arange(len(dst))
    isA = half_flag == 0
    # rank within (block, half)
    rank = idxs - np.concatenate(
        [np.repeat(0, 0), np.zeros(len(dst), np.int64)])
    # compute rank via per-block offsets
    block_start = np.searchsorted(blk, np.arange(nblk))
    rank_in_block = idxs - block_start[blk]
    rankA = rank_in_block                     # A edges come first in block
    rankB = rank_in_block - cA[blk]
    pos = np.where(isA,
                   blk * (CH2 * 128) + rankA,
                   blk * (CH2 * 128) + CHH * 128 + rankB)
    psrc[pos] = np.where(isA, src, src - half)
    prel[pos] = dst & 127
    pdstl[pos] = dst % shard

    dead = half                               # dead row index in each half
    psrc[psrc < 0] = dead

    psrc = psrc.reshape(N_CORES, NB, CH2, 128)
    prel = prel.reshape(N_CORES, NB, CH2, 128)
    pdstl = pdstl.reshape(N_CORES, NB, CH2, 128)

    packs = []
    for c in range(N_CORES):
        sc_ = psrc[c]
        idsA = sc_[:, :CHH, :].reshape(-1)    # [NB*CHH*128] A-stream
        idsB = sc_[:, CHH:, :].reshape(-1)
        idsD = pdstl[c].reshape(-1)           # [NB*CH2*128] dst stream
        relc = np.ascontiguousarray(
            prel[c].transpose(2, 0, 1).reshape(128, NB * CH2)
            .astype(np.float32))              # [128, NB*CH2]
        packs.append({
            "zidxA": _wrap(idsA), "zidxB": _wrap(idsB),
            "adidx": _wrap(idsD), "relD": relc,
        })
    return CHH, packs


# ---------------------------------------------------------------------------
# device program
# ---------------------------------------------------------------------------

def build_program(NB, CHH, layers, DB, reps=1):
    shard = NB * 128
    n_pad = N_CORES * shard
    half = n_pad // 2
    nwin = n_pad // 128
    nlay = len(layers)
    CH2 = 2 * CHH
    assert nwin % DB == 0 and (nwin // 2) % DB == 0

    nc = bacc.Bacc("TRN2", target_bir_lowering=False, debug=False,
                   num_devices=N_CORES)
    fi0 = layers[0][0]
    xT1 = nc.dram_tensor("xT1", [fi0, n_pad], F32, kind="ExternalInput").ap()
    xTmy = nc.dram_tensor("xTmy", [fi0, shard], F32, kind="ExternalInput").ap()
    wcat, wad, brep, deadr = [], [], [], []
    for l, (fi, fo) in enumerate(layers):
        wcat.append(nc.dram_tensor(f"wcat{l}", [fi, fo + 4], F32,
                                   kind="ExternalInput").ap())
        wad.append(nc.dram_tensor(f"wad{l}", [fi, 4], F32,
                                  kind="ExternalInput").ap())
        brep.append(nc.dram_tensor(f"brep{l}", [128, fo], F32,
                                   kind="ExternalInput").ap())
        deadr.append(nc.dram_tensor(f"dead{l}", [1, _we(fo)], F32,
                                    kind="ExternalInput").ap())
    nA = NB * CHH * 128
    nD = NB * CH2 * 128
    zidxA = nc.dram_tensor("zidxA", [128, nA // 16], I16,
                           kind="ExternalInput").ap()
    zidxB = nc.dram_tensor("zidxB", [128, nA // 16], I16,
                           kind="ExternalInput").ap()
    adidx = nc.dram_tensor("adidx", [128, nD // 16], I16,
                           kind="ExternalInput").ap()
    relD = nc.dram_tensor("relD", [128, NB * CH2], F32,
                          kind="ExternalInput").ap()
    iota_in = nc.dram_tensor("iota", [128, 128], F32, kind="ExternalInput").ap()
    ident_in = nc.dram_tensor("ident", [128, 128], F32,
                              kind="ExternalInput").ap()
    wr_in = nc.dram_tensor("wr", [128, 2, 32], F32, kind="ExternalInput").ap()
    wf1_in = nc.dram_tensor("wf1", [32, 64], F32, kind="ExternalInput").ap()
    wf2_in = nc.dram_tensor("wf2", [64, 1], F32, kind="ExternalInput").ap()
    br_in = nc.dram_tensor("brv", [32, 1], F32, kind="ExternalInput").ap()
    bf1_in = nc.dram_tensor("bf1v", [64, 1], F32, kind="ExternalInput").ap()
    bf2_in = nc.dram_tensor("bf2v", [1, 1], F32, kind="ExternalInput").ap()
    out_t = nc.dram_tensor("out", [NB, 128], F32, kind="ExternalOutput").ap()

    zA, zB, adS, agin, agx = [], [], [], [], []
    for l, (fi, fo) in enumerate(layers):
        WE = _we(fo)
        zA.append(nc.dram_tensor(f"zA{l}", [half + 1, WE], F32).ap())
        zB.append(nc.dram_tensor(f"zB{l}", [half + 1, WE], F32).ap())
        adS.append(nc.dram_tensor(f"adS{l}", [shard, 64], F32).ap())
        if l < nlay - 1:
            agin.append(nc.dram_tensor(f"agin{l}", [fo, shard], F32).ap())
            agx.append(nc.dram_tensor(f"agx{l}", [N_CORES, fo, shard],
                                      F32, addr_space="Shared").ap())
        else:
            agin.append(None)
            agx.append(None)

    AT = mybir.ActivationFunctionType
    OP = mybir.AluOpType

    with tile.TileContext(nc) as tc, ExitStack() as ctx:
        cpool = ctx.enter_context(tc.tile_pool(name="consts", bufs=1))
        dlhs = ctx.enter_context(tc.tile_pool(name="dlhs", bufs=3))
        dbat = ctx.enter_context(tc.tile_pool(name="dbat", bufs=3))
        gA_p = ctx.enter_context(tc.tile_pool(name="gA", bufs=3))
        gB_p = ctx.enter_context(tc.tile_pool(name="gB", bufs=3))
        gD_p = ctx.enter_context(tc.tile_pool(name="gD", bufs=3))
        epool = ctx.enter_context(tc.tile_pool(name="edge_small", bufs=3))
        dmat_p = ctx.enter_context(tc.tile_pool(name="dmat", bufs=2))
        ypool = ctx.enter_context(tc.tile_pool(name="epi", bufs=2))
        ps_d = ctx.enter_context(tc.tile_pool(name="ps_dense", bufs=2,
                                              space="PSUM"))
        ps_e = ctx.enter_context(tc.tile_pool(name="ps_edge", bufs=2,
                                              space="PSUM"))
        ps_t = ctx.enter_context(tc.tile_pool(name="ps_tr", bufs=2,
                                              space="PSUM"))

        def csb(ap_in, shape, dtype=F32, tag=None):
            t = cpool.tile(shape, dtype, tag=tag or ap_in.tensor.name)
            nc.sync.dma_start(t[:], ap_in[:])
            return t

        iota_sb = csb(iota_in, [128, 128])
        ident_sb = csb(ident_in, [128, 128])
        zidxA_sb = csb(zidxA, [128, nA // 16], I16)
        zidxB_sb = csb(zidxB, [128, nA // 16], I16)
        adidx_sb = csb(adidx, [128, nD // 16], I16)
        relD_sb = csb(relD, [128, NB * CH2])
        wr_sb = csb(wr_in, [128, 2, 32])
        wf1_sb = csb(wf1_in, [32, 64])
        wf2_sb = csb(wf2_in, [64, 1])
        br_sb = csb(br_in, [32, 1])
        bf1_sb = csb(bf1_in, [64, 1])
        bf2_sb = csb(bf2_in, [1, 1])
        wcat_sb = [csb(wcat[l], [layers[l][0], layers[l][1] + 4])
                   for l in range(nlay)]
        wad_sb = [csb(wad[l], [layers[l][0], 4]) for l in range(nlay)]
        brep_sb = [csb(brep[l], [128, layers[l][1]]) for l in range(nlay)]

        def elu_chain(y_ap, shape, tag):
            m = epool.tile(shape, F32, tag=f"elu_m_{tag}")
            nc.vector.tensor_scalar(m[:], y_ap, 0.0, None, OP.min)
            e = epool.tile(shape, F32, tag=f"elu_e_{tag}")
            nc.scalar.activation(e[:], m[:], AT.Exp)
            nc.vector.tensor_scalar(y_ap, y_ap, 0.0, -1.0, OP.max, OP.add)
            nc.vector.tensor_tensor(y_ap, y_ap, e[:], op=OP.add)

        for _rep in range(reps):
          for l, (fi, fo) in enumerate(layers):
            WE = _we(fo)
            Wz = fo + 4
            C = fo // H
            nfc = (fo + 127) // 128

            # ---------------- dense (replicated, writes zA/zB) ----------
            for wb in range(0, nwin, DB):
                lx = dlhs.tile([fi, DB * 128], F32, tag="lx")
                if l == 0:
                    nc.sync.dma_start(lx[:], xT1[:, wb * 128:(wb + DB) * 128])
                else:
                    s = (wb * 128) // shard
                    woff = wb * 128 - s * shard
                    nc.sync.dma_start(
                        lx[:], agx[l - 1][s, :, woff:woff + DB * 128])
                zb = dbat.tile([128, DB, Wz], F32, tag="zb")
                for j in range(DB):
                    zp = ps_d.tile([128, Wz], F32, tag="zp")
                    nc.tensor.matmul(out=zp[:], lhsT=lx[:, j * 128:(j + 1) * 128],
                                     rhs=wcat_sb[l][:], start=True, stop=True)
                    if j % 2 == 0:
                        nc.vector.tensor_copy(zb[:, j, :], zp[:])
                    else:
                        nc.scalar.activation(zb[:, j, :], zp[:], AT.Copy)
                ztgt = zA[l] if wb < nwin // 2 else zB[l]
                roff = wb * 128 if wb < nwin // 2 else (wb - nwin // 2) * 128
                dst_ap = _mk_ap(ztgt.tensor, roff * WE,
                                [(WE, 128), (128 * WE, DB), (1, Wz)])
                nc.sync.dma_start(dst_ap, zb[:, :, :])
            nc.gpsimd.dma_start(zA[l][half:half + 1, :], deadr[l][:])
            nc.gpsimd.dma_start(zB[l][half:half + 1, :], deadr[l][:])

            # ---------------- mini-dense: alpha_dst for MY shard --------
            for wb in range(0, NB, DB):
                lx = dlhs.tile([fi, DB * 128], F32, tag="lxmy")
                if l == 0:
                    nc.sync.dma_start(lx[:], xTmy[:, wb * 128:(wb + DB) * 128])
                else:
                    nc.sync.dma_start(
                        lx[:], agin[l - 1][:, wb * 128:(wb + DB) * 128])
                ab = dbat.tile([128, DB, 4], F32, tag="ab")
                for j in range(DB):
                    ap_ = ps_d.tile([128, 4], F32, tag="zp")
                    nc.tensor.matmul(out=ap_[:], lhsT=lx[:, j * 128:(j + 1) * 128],
                                     rhs=wad_sb[l][:], start=True, stop=True)
                    nc.vector.tensor_copy(ab[:, j, :], ap_[:])
                dst_ap = _mk_ap(adS[l].tensor, wb * 128 * 64,
                                [(64, 128), (128 * 64, DB), (1, 4)])
                nc.sync.dma_start(dst_ap, ab[:, :, :])

            # ---------------- edge phase --------------------------------
            zwinA, zwinB, dwin = {}, {}, {}

            def zchunk(stream, pos):
                tabs, idxs, wins, pool, we, tagp = (
                    (zA[l], zidxA_sb, zwinA, gA_p, WE, "wa")
                    if stream == "A" else
                    (zB[l], zidxB_sb, zwinB, gB_p, WE, "wb")
                    if stream == "B" else
                    (adS[l], adidx_sb, dwin, gD_p, 64, "wd"))
                w, slot = pos // WCH, pos % WCH
                if w not in wins:
                    total = nA // 128 if stream in "AB" else nD // 128
                    ni = min(WNI, (total - w * WCH) * 128)
                    t = pool.tile([128, WCH, we], F32, tag=tagp)
                    nc.gpsimd.dma_gather(
                        out_ap=t[:, :ni // 128, :], in_ap=tabs[:],
                        idxs_ap=idxs[:, 64 * w: 64 * w + ni // 16],
                        num_idxs=ni, num_idxs_reg=ni, elem_size=we)
                    wins[w] = t
                return wins[w], slot

            for b in range(NB):
                # gather tiles + segment list (contiguous runs in same tiles)
                chunks = []
                for j in range(CH2):
                    st = "A" if j < CHH else "B"
                    zpos = b * CHH + (j % CHH)
                    zt, zs = zchunk(st, zpos)
                    dt_, ds = zchunk("D", b * CH2 + j)
                    chunks.append((zt, zs, dt_, ds))
                segs = []
                j0 = 0
                for j in range(1, CH2 + 1):
                    if (j == CH2 or chunks[j][0] is not chunks[j - 1][0]
                            or chunks[j][1] != chunks[j - 1][1] + 1
                            or chunks[j][2] is not chunks[j - 1][2]
                            or chunks[j][3] != chunks[j - 1][3] + 1):
                        segs.append((j0, j))
                        j0 = j
                # scores ee = exp(max(s, .2s)) written into alpha_s cols
                for (a, bb) in segs:
                    ns = bb - a
                    zt, zs, dt_, ds = chunks[a]
                    asv = zt[:, zs:zs + ns, fo:Wz]
                    adv = dt_[:, ds:ds + ns, 0:4]
                    sc = epool.tile([128, WCH, 4], F32, tag="sc")
                    nc.vector.tensor_tensor(sc[:, :ns, :], asv, adv, op=OP.add)
                    sc2 = epool.tile([128, WCH, 4], F32, tag="sc2")
                    nc.vector.tensor_scalar(sc2[:, :ns, :], sc[:, :ns, :],
                                            0.2, None, OP.mult)
                    nc.vector.tensor_tensor(sc[:, :ns, :], sc[:, :ns, :],
                                            sc2[:, :ns, :], op=OP.max)
                    nc.scalar.activation(asv, sc[:, :ns, :], AT.Exp)
                    # msg scaling in-place on z cols
                    for hf in range(nfc):
                        h0 = hf * 128 // C
                        nh = min(H - h0, 128 // C)
                        cols = zt[:, zs:zs + ns, hf * 128:hf * 128 + nh * C]
                        g4 = bass.AP(cols.tensor, cols.offset,
                                     [cols.ap[0], cols.ap[1], [C, nh], [1, C]])
                        ee4 = asv[:, :, h0:h0 + nh].unsqueeze(3).to_broadcast(
                            [128, ns, nh, C])
                        nc.vector.tensor_tensor(g4, g4, ee4, op=OP.mult)
                # one-hot for the whole block
                dm = dmat_p.tile([128, CH2, 128], F32, tag="dm")
                rel = relD_sb[:, b * CH2:(b + 1) * CH2]
                nc.vector.tensor_tensor(
                    dm[:], rel.unsqueeze(2).to_broadcast([128, CH2, 128]),
                    iota_sb[:].unsqueeze(1).to_broadcast([128, CH2, 128]),
                    op=OP.is_equal)
                # aggregation
                pb = ps_e.tile([128, Wz], F32, tag="pb")
                for j in range(CH2):
                    zt, zs, _, _ = chunks[j]
                    nc.tensor.matmul(out=pb[:], lhsT=dm[:, j, :],
                                     rhs=zt[:, zs, 0:Wz], start=(j == 0),
                                     stop=(j == CH2 - 1))
                # epilogue: x = elu(num/den + b)
                r4 = epool.tile([128, 4], F32, tag="r4")
                nc.vector.reciprocal(r4[:], pb[:, fo:Wz])
                y = ypool.tile([128, fo], F32, tag="y")
                y3 = bass.AP(y[:].tensor, y[:].offset,
                             [y[:].ap[0], [C, H], [1, C]])
                pb3 = bass.AP(pb[:].tensor, pb[:].offset,
                              [pb[:].ap[0], [C, H], [1, C]])
                nc.vector.tensor_tensor(
                    y3, pb3, r4[:].unsqueeze(2).to_broadcast([128, H, C]),
                    op=OP.mult)
                nc.vector.tensor_tensor(y[:], y[:], brep_sb[l][:, :], op=OP.add)
                elu_chain(y[:], [128, fo], "blk")
                xbt = []
                for fc in range(nfc):
                    ncol = min(128, fo - fc * 128)
                    tp = ps_t.tile([128, 128], F32, tag="tp")
                    nc.tensor.transpose(
                        out=tp[:ncol, :], in_=y[:, fc * 128:fc * 128 + ncol],
                        identity=ident_sb[:])
                    xt = ypool.tile([128, 128], F32, tag=f"xt{fc}")
                    if fc % 2 == 0:
                        nc.vector.tensor_copy(xt[:ncol, :], tp[:ncol, :])
                    else:
                        nc.scalar.activation(xt[:ncol, :], tp[:ncol, :], AT.Copy)
                    xbt.append((xt, ncol))
                if l < nlay - 1:
                    for fc, (xt, ncol) in enumerate(xbt):
                        nc.scalar.dma_start(
                            agin[l][fc * 128:fc * 128 + ncol,
                                    b * 128:(b + 1) * 128], xt[:ncol, :])
                else:
                    r1p = ps_t.tile([32, 128], F32, tag="mlp")
                    for fc, (xt, ncol) in enumerate(xbt):
                        nc.tensor.matmul(out=r1p[:], lhsT=wr_sb[:ncol, fc, :],
                                         rhs=xt[:ncol, :], start=(fc == 0),
                                         stop=(fc == len(xbt) - 1))
                    r1 = ypool.tile([32, 128], F32, tag="r1")
                    nc.vector.tensor_scalar(r1[:], r1p[:], br_sb[:, 0:1],
                                            None, OP.add)
                    elu_chain(r1[:], [32, 128], "mlp1")
                    r2p = ps_t.tile([64, 128], F32, tag="mlp")
                    nc.tensor.matmul(out=r2p[:], lhsT=wf1_sb[:], rhs=r1[:],
                                     start=True, stop=True)
                    r2 = ypool.tile([64, 128], F32, tag="r2")
                    nc.vector.tensor_scalar(r2[:], r2p[:], bf1_sb[:, 0:1],
                                            None, OP.add)
                    elu_chain(r2[:], [64, 128], "mlp2")
                    op_ = ps_t.tile([1, 128], F32, tag="mlp")
                    nc.tensor.matmul(out=op_[:], lhsT=wf2_sb[:], rhs=r2[:],
                                     start=True, stop=True)
                    ob = ypool.tile([1, 128], F32, tag="ob")
                    nc.vector.tensor_scalar(ob[:], op_[:], bf2_sb[:, 0:1],
                                            None, OP.add)
                    nc.sync.dma_start(out_t[b:b + 1, :], ob[:])

            if l < nlay - 1:
                nc.gpsimd.collective_compute(
                    "AllGather", OP.bypass,
                    replica_groups=[list(range(N_CORES))],
                    ins=[agin[l][:]], outs=[agx[l][:]])

    nc.compile()
    return nc


# ---------------------------------------------------------------------------
# host orchestration
# ---------------------------------------------------------------------------

def _prep_weights(layers, W, A_s, A_d, B):
    wcat, wadl, brep, dead = [], [], [], []
    for l, (fi, fo) in enumerate(layers):
        C = fo // H
        As = np.zeros((fo, 4), np.float32)
        Ad = np.zeros((fo, 4), np.float32)
        for h in range(H):
            As[h * C:(h + 1) * C, h] = A_s[l][h]
            Ad[h * C:(h + 1) * C, h] = A_d[l][h]
        wcat.append(np.ascontiguousarray(
            np.concatenate([W[l], W[l] @ As], axis=1), np.float32))
        wadl.append(np.ascontiguousarray(W[l] @ Ad, np.float32))
        brep.append(np.ascontiguousarray(
            np.tile(B[l][None, :], (128, 1)), np.float32))
        d = np.zeros((1, _we(fo)), np.float32)
        d[0, fo:fo + 4] = NEG
        dead.append(d)
    return wcat, wadl, brep, dead


_CACHE = {}


def run_gat(x, edge_index, W, A_s, A_d, B, Wr, br, Wf1, bf1, Wf2, bf2,
            layers, N, NB, DB=7, reps=1):
    shard = NB * 128
    n_pad = N_CORES * shard
    CHH, packs = build_structure(edge_index, N, NB)

    key = (NB, CHH, tuple(layers), DB, reps)
    if key not in _CACHE:
        t0 = time.time()
        _CACHE[key] = build_program(NB, CHH, layers, DB=DB, reps=reps)
        print(f"[kernel] built+compiled program NB={NB} CHH={CHH} "
              f"in {time.time() - t0:.1f}s", flush=True)
    nc = _CACHE[key]

    wcat, wadl, brep, dead = _prep_weights(layers, W, A_s, A_d, B)
    xp = np.zeros((n_pad, layers[0][0]), np.float32)
    xp[:N] = x
    xT1 = np.ascontiguousarray(xp.T)
    base = {
        "xT1": xT1,
        "iota": np.ascontiguousarray(
            np.tile(np.arange(128, dtype=np.float32)[None, :], (128, 1))),
        "ident": np.eye(128, dtype=np.float32),
        "wr": np.ascontiguousarray(
            Wr.astype(np.float32).reshape(2, 128, 32).transpose(1, 0, 2)),
        "wf1": Wf1.astype(np.float32), "wf2": Wf2.astype(np.float32),
        "brv": br.reshape(32, 1).astype(np.float32),
        "bf1v": bf1.reshape(64, 1).astype(np.float32),
        "bf2v": bf2.reshape(1, 1).astype(np.float32),
    }
    for l in range(len(layers)):
        base[f"wcat{l}"] = wcat[l]
        base[f"wad{l}"] = wadl[l]
        base[f"brep{l}"] = brep[l]
        base[f"dead{l}"] = dead[l]
    in_maps = []
    for c in range(N_CORES):
        m = dict(base)
        m.update(packs[c])
        m["xTmy"] = np.ascontiguousarray(xT1[:, c * shard:(c + 1) * shard])
        in_maps.append(m)

    res = run_bass_kernel_spmd(nc, in_maps, list(range(N_CORES)))
    outs = [res.results[c]["out"].reshape(-1) for c in range(N_CORES)]
    full = np.concatenate(outs)[:N]
    return full.reshape(N, 1).astype(np.float32)


def kernel(**inputs):
    x = np.asarray(inputs["x"], np.float32)
    edge_index = np.asarray(inputs["edge_index"], np.int32)
    W = [np.asarray(inputs[f"W{l+1}"], np.float32) for l in range(4)]
    A_s = [np.asarray(inputs[f"as{l+1}"], np.float32) for l in range(4)]
    A_d = [np.asarray(inputs[f"ad{l+1}"], np.float32) for l in range(4)]
    B = [np.asarray(inputs[f"b{l+1}"], np.float32) for l in range(4)]
    return run_gat(
        x, edge_index, W, A_s, A_d, B,
        np.asarray(inputs["Wr"], np.float32), np.asarray(inputs["br"], np.float32),
        np.asarray(inputs["Wf1"], np.float32), np.asarray(inputs["bf1"], np.float32),
        np.asarray(inputs["Wf2"], np.float32), np.asarray(inputs["bf2"], np.float32),
        FULL_LAYERS, FULL_N, NB=49)
